# revision 45
# baseline (speedup 1.0000x reference)
"""Trainium2 Bass kernel for nn_FAM_82463372083966.

Reference computation (per batch b of 4, C=256 channels, N=4096 positions):
    qk   = qk_w @ x                     [32, N]
    x_q  = relu(bn1(qk)).T              [N, 32]
    x_k  = relu(bn2(qk))                [32, N]
    x_v  = relu(bn3(v_w @ x + v_b))     [C, N]
    energy = x_q @ x_k - rowsum(x_q) * colsum(x_k)   [N, N]
    att  = softmax(energy, axis=-1);  att /= (1e-9 + att.sum(axis=0))
    x_r  = x_v @ att                    [C, N]
    out  = x + relu(an(trans_w @ (x - x_r) + trans_b))

Key identities used:
  * energy = x_q^T (I - 11^T) x_k  -> subtract channel-colsums from x_k once
    (k~), pure K=32 contraction.
  * softmax without max-subtraction is safe here with a constant +60 shift
    (energies are all negative, per-row maxima in [-90, 0)); the shift
    cancels exactly in the normalization.
  * row-normalization (1/rowsumexp) is folded into the x_v^T operand;
    appending a ones column to x_v^T makes the attention column-sums
    (L1-renorm denominators) fall out of the same matmul.

Sharding over 8 cores: core = (b, h) = (batch, column half of the N x N
attention). Each core computes P = exp(energy+60) for ALL rows x its own
2048 columns (stored bf16 in SBUF), so the x_v @ att contraction is fully
local. The only cross-core exchange is the row-sum denominators: four
pipelined [128,8] f32 AllReduces between the two cores sharing a batch,
each covering 8 row-blocks so the next group's exp work hides the latency.

Schedule (per core): phase 1 computes projections (x_q, k~, x_v^T with the
BN/bias/normalization folds); the fused phase 2 streams 32 row-blocks of
energy matmuls -> exp (ACT-bound), interleaving the first column-group's
numerator matmuls into the exp shadows one collective-group behind; the
tail pass finishes the second column-group + attention column-sums
(PE-dense); finalize divides, applies the trans conv + residual, all
emitted round-robin so the in-order engines pipeline.
"""
import numpy as np

import concourse.bass as bass
import concourse.mybir as mybir
import concourse.tile as tile
from concourse.tile_rust import add_dep_helper
from concourse import bacc, bass_isa
from concourse.bass_utils import run_bass_kernel_spmd

F32 = mybir.dt.float32
F32R = mybir.dt.float32r
BF16 = mybir.dt.bfloat16
AF = mybir.ActivationFunctionType

B, C, N, CR = 4, 256, 4096, 32
NH = N // 2          # columns per core
RB = N // 128        # 32 row blocks
MC = NH // 512       # 4 column chunks of 512
EXP_SHIFT = 60.0

REPLICA_PAIRS = [[0, 1], [2, 3], [4, 5], [6, 7]]


def finalize_chains(nc, mg, nm_src, nmc_src, spool, x_loc):
    """rc broadcast + diff = x - nm*rc for one 1024-col group. nm_src(i, ssl)
    returns an AP slice (SBUF or PSUM); nmc_src() the [1,1024] colsum AP.
    Only the 4 diff tiles are held; x reloads at the residual step."""
    rcs_t, df_t = {}, {}
    for sub in range(2):
        ssl = slice(512 * sub, 512 * (sub + 1))
        rc0 = spool.tile([1, 512], F32, name="rc0", tag="rc0", bufs=2)
        nc.vector.tensor_scalar_add(rc0[:], nmc_src()[:, ssl], 1e-9)
        nc.vector.reciprocal(rc0[:], rc0[:])
        rcs = spool.tile([128, 512], F32, name="rcs", tag="rcs", bufs=2)
        nc.gpsimd.partition_broadcast(rcs[:], rc0[:])
        rcs_t[sub] = rcs
    for sub in range(2):
        msl = slice(1024 * mg + 512 * sub, 1024 * mg + 512 * (sub + 1))
        ssl = slice(512 * sub, 512 * (sub + 1))
        for i in range(2):
            t = spool.tile([128, 512], F32, name=f"cxl{i}", tag=f"cxl{i}", bufs=2)
            nc.sync.dma_start(t[:], x_loc[128 * i:128 * (i + 1), msl])
            d = spool.tile([128, 512], F32R, name=f"df{i}", tag=f"df{i}", bufs=3)
            nc.vector.tensor_mul(d[:], nm_src(i, ssl), rcs_t[sub][:])
            nc.vector.tensor_sub(d[:], t[:], d[:])
            df_t[(sub, i)] = d
    return df_t


def finalize_tf(nc, mg, df_t, ppool, spool, x_loc, out_loc, w3t, bias3):
    """transform + residual + store for one 1024-col group."""
    xl_t = {}
    for sub in range(2):
        msl = slice(1024 * mg + 512 * sub, 1024 * mg + 512 * (sub + 1))
        for i in range(2):
            t = spool.tile([128, 512], F32, name=f"rxl{i}", tag=f"rxl{i}", bufs=2)
            nc.sync.dma_start(t[:], x_loc[128 * i:128 * (i + 1), msl])
            xl_t[(sub, i)] = t
    for sub in range(2):
        msl = slice(1024 * mg + 512 * sub, 1024 * mg + 512 * (sub + 1))
        for mo in range(2):
            osl = slice(128 * mo, 128 * (mo + 1))
            tf = ppool.tile([128, 512], F32, name=f"tf{mo}", tag="tf", bufs=2)
            nc.tensor.matmul(tf[:], w3t[0][:, osl], df_t[(sub, 0)][:], start=True, stop=False)
            nc.tensor.matmul(tf[:], w3t[1][:, osl], df_t[(sub, 1)][:], start=False, stop=True)
            t2 = spool.tile([128, 512], F32, name=f"t2{mo}", tag="t2", bufs=2)
            nc.scalar.activation(t2[:], tf[:], AF.Relu, bias=bias3[:, mo:mo + 1])
            ot = spool.tile([128, 512], F32, name=f"ot{mo}", tag="ot", bufs=2)
            if mo == 0:
                nc.vector.tensor_add(ot[:], xl_t[(sub, mo)][:], t2[:])
            else:
                nc.gpsimd.tensor_add(ot[:], xl_t[(sub, mo)][:], t2[:])
            nc.sync.dma_start(out_loc[osl, msl], ot[:])


def build_nc(reps=1, collective=True):
    nc = bacc.Bacc("TRN2", target_bir_lowering=False, debug=False, num_devices=8)

    # ---- I/O -------------------------------------------------------------
    x_full = nc.declare_dram_parameter("x_full", [C, N], F32, isOutput=False)
    x_loc = nc.declare_dram_parameter("x_loc", [C, NH], F32, isOutput=False)
    qkwT_d = nc.declare_dram_parameter("qkwT", [C, CR], F32, isOutput=False)
    w2t_d = nc.declare_dram_parameter("w2t", [C, 258], F32, isOutput=False)
    b2e_d = nc.declare_dram_parameter("b2e", [1, 258], F32, isOutput=False)
    w3t_d = nc.declare_dram_parameter("w3t", [C, C], F32, isOutput=False)
    bias3_d = nc.declare_dram_parameter("bias3", [128, 2], F32, isOutput=False)
    bn1s_d = nc.declare_dram_parameter("bn1s", [CR, 1], F32, isOutput=False)
    bn1b_d = nc.declare_dram_parameter("bn1b", [CR, 1], F32, isOutput=False)
    bn2s_d = nc.declare_dram_parameter("bn2s", [CR, 1], F32, isOutput=False)
    bn2b_d = nc.declare_dram_parameter("bn2b", [CR, 1], F32, isOutput=False)
    ones_d = nc.declare_dram_parameter("ones1", [1, 128], F32, isOutput=False)
    out_loc = nc.declare_dram_parameter("out_loc", [C, NH], F32, isOutput=True)

    NG = 4               # collective groups
    GRB = RB // NG       # row blocks per group (8)
    rs_in = [nc.dram_tensor(f"rs_in{g}", [128, GRB], F32) for g in range(NG)]
    nm_d = [[nc.dram_tensor(f"nm_d{m}{i}", [128, 1024], F32) for i in range(2)] for m in range(2)]
    nmc_d = [nc.dram_tensor(f"nmc_d{m}", [1, 1024], F32) for m in range(2)]
    rs_out = [nc.dram_tensor(f"rs_out{g}", [128, GRB], F32) for g in range(NG)]

    with tile.TileContext(nc) as tc:
        with (
            tc.tile_pool(name="const", bufs=1) as cp,
            tc.tile_pool(name="main", bufs=1) as mp,
        ):
            # ---- constants ----------------------------------------------
            qkwT = [cp.tile([128, CR], F32R, name=f"qkwT{i}", tag=f"qkwT{i}") for i in range(2)]
            w2t = [cp.tile([128, 258], F32R, name=f"w2t{i}", tag=f"w2t{i}") for i in range(2)]
            b2e = cp.tile([1, 258], BF16, name="b2e", tag="b2e")
            b2ef = cp.tile([1, 258], F32, name="b2ef", tag="b2ef")
            ones_bf = cp.tile([1, 128], BF16, name="ones_bf", tag="ones_bf")
            w3t = [cp.tile([128, C], F32R, name=f"w3t{i}", tag=f"w3t{i}") for i in range(2)]
            bias3 = cp.tile([128, 2], F32, name="bias3", tag="bias3")
            bn1s = cp.tile([CR, 1], F32, name="bn1s", tag="bn1s")
            bn1b = cp.tile([CR, 1], F32, name="bn1b", tag="bn1b")
            bn2s = cp.tile([CR, 1], F32, name="bn2s", tag="bn2s")
            bn2b = cp.tile([CR, 1], F32, name="bn2b", tag="bn2b")
            ones1 = cp.tile([1, 128], F32R, name="ones1", tag="ones1")
            e60 = cp.tile([128, 1], F32, name="e60", tag="e60")
            nc.vector.memset(e60[:], EXP_SHIFT)
            for i in range(2):
                nc.sync.dma_start(qkwT[i][:], qkwT_d[128 * i:128 * (i + 1), :].bitcast(F32R))
            for i in range(2):
                nc.sync.dma_start(w2t[i][:], w2t_d[128 * i:128 * (i + 1), :].bitcast(F32R))
                nc.sync.dma_start(w3t[i][:], w3t_d[128 * i:128 * (i + 1), :].bitcast(F32R))
            nc.sync.dma_start(b2ef[:], b2e_d[:])
            nc.vector.tensor_copy(b2e[:], b2ef[:])
            nc.sync.dma_start(bias3[:], bias3_d[:])
            nc.sync.dma_start(bn1s[:], bn1s_d[:])
            nc.sync.dma_start(bn1b[:], bn1b_d[:])
            nc.sync.dma_start(bn2s[:], bn2s_d[:])
            nc.sync.dma_start(bn2b[:], bn2b_d[:])
            nc.sync.dma_start(ones1[:], ones_d[:].bitcast(F32R))
            nc.vector.tensor_copy(ones_bf[:], ones1[:].bitcast(F32))

            # ---- persistent state ---------------------------------------
            rowsum = mp.tile([128, RB], F32, name="rowsum", tag="rowsum")
            rsf = mp.tile([128, RB], F32, name="rsf", tag="rsf")
            recips = mp.tile([128, RB], F32, name="recips", tag="recips")

            for _rep in range(reps):
              with tc.tile_pool(name="pP", bufs=1) as pP:
                xvt = [pP.tile([128, 258], BF16, name=f"xvt{nb}", tag=f"xvt{nb}") for nb in range(RB)]
                ptile = [pP.tile([128, NH], BF16, name=f"P{rb}", tag=f"P{rb}") for rb in range(RB)]
                with tc.tile_pool(name="qkA", bufs=1) as qkA:
                    xq = qkA.tile([CR, N], F32R, name="xq", tag="xq")
                    ktl = qkA.tile([CR, NH], F32R, name="ktl", tag="ktl")
                    # ---- phase 1: projections ----------------------------
                    with (
                        tc.tile_pool(name="s1", bufs=2) as s1,
                        tc.tile_pool(name="p1", bufs=2, space="PSUM") as p1,
                    ):
                        for c8 in range(8):
                            sl = slice(512 * c8, 512 * (c8 + 1))
                            xc = []
                            for i in range(2):
                                t = s1.tile([128, 512], F32R, name=f"xc{i}", tag=f"xc{i}", bufs=3)
                                nc.sync.dma_start(t[:], x_full[128 * i:128 * (i + 1), sl].bitcast(F32R))
                                xc.append(t)
                            qk_ps = p1.tile([CR, 512], F32, name="qk", tag="qk")
                            nc.tensor.matmul(qk_ps[:], qkwT[0][:], xc[0][:], start=True, stop=False)
                            nc.tensor.matmul(qk_ps[:], qkwT[1][:], xc[1][:], start=False, stop=True)
                            nc.scalar.activation(xq[:, sl], qk_ps[:], AF.Relu, bias=bn1b[:], scale=bn1s[:])
                            for j in range(4):
                                nb = 4 * c8 + j
                                jsl = slice(128 * j, 128 * (j + 1))
                                xv_ps = p1.tile([128, 258], F32, name="xv", tag="xv")
                                nc.tensor.matmul(xv_ps[:], xc[0][:, jsl], w2t[0][:], start=True, stop=False)
                                nc.tensor.matmul(xv_ps[:], xc[1][:, jsl], w2t[1][:], start=False, stop=False)
                                nc.tensor.matmul(xv_ps[:], ones_bf[:], b2e[:], start=False, stop=True)
                                nc.vector.tensor_scalar_max(xvt[nb][:], xv_ps[:], 0.0)
                        for c4 in range(4):
                            sl = slice(512 * c4, 512 * (c4 + 1))
                            xlc = []
                            for i in range(2):
                                t = s1.tile([128, 512], F32R, name=f"xl{i}", tag=f"xl{i}", bufs=2)
                                nc.sync.dma_start(t[:], x_loc[128 * i:128 * (i + 1), sl].bitcast(F32R))
                                xlc.append(t)
                            qk_ps = p1.tile([CR, 512], F32, name="qk2", tag="qk")
                            nc.tensor.matmul(qk_ps[:], qkwT[0][:], xlc[0][:], start=True, stop=False)
                            nc.tensor.matmul(qk_ps[:], qkwT[1][:], xlc[1][:], start=False, stop=True)
                            xk_c = s1.tile([CR, 512], F32, name="xk", tag="xk")
                            nc.scalar.activation(xk_c[:], qk_ps[:], AF.Relu, bias=bn2b[:], scale=bn2s[:])
                            ks_c = s1.tile([CR, 512], F32, name="ks", tag="ks")
                            nc.gpsimd.partition_all_reduce(ks_c[:], xk_c[:], channels=CR,
                                                           reduce_op=bass_isa.ReduceOp.add)
                            nc.vector.tensor_sub(ktl[:, sl], xk_c[:], ks_c[:])

                    # ---- phase 2 (fused): energy+exp+nm(col group 0) -----
                    with (
                        tc.tile_pool(name="stA", bufs=1) as stA,
                        tc.tile_pool(name="pA", bufs=1, space="PSUM") as pA,
                    ):
                        pEn_cm = tc.tile_pool(name="pEn", bufs=1, space="PSUM")
                        pEn = pEn_cm.__enter__()
                        nm0 = [pA.tile([128, 1024], F32, name=f"g0nm{i}", tag=f"g0nm{i}") for i in range(2)]

                        # queue of single numer MMs (rb, chunk); one per
                        # exp half-slot so each 427ns MM fits the ~600ns
                        # PE shadow under a 1038ns exp
                        queue = []

                        def emit_one(rb, i, hf):
                            st, sp = rb == 0, rb == RB - 1
                            osl = slice(512 * hf, 512 * (hf + 1))
                            nc.tensor.matmul(nm0[i][:, osl], xvt[rb][:, 128 * i:128 * (i + 1)],
                                             ptile[rb][:, osl], start=st, stop=sp)

                        for g in range(NG):
                            for rb in range(GRB * g, GRB * (g + 1)):
                                for h in range(2):
                                    en = pEn.tile([128, 1024], F32, name="en", tag="en", bufs=2)
                                    for q in range(2):
                                        qsl = slice(1024 * h + 512 * q, 1024 * h + 512 * (q + 1))
                                        nc.tensor.matmul(en[:, 512 * q:512 * (q + 1)],
                                                         xq[:, 128 * rb:128 * (rb + 1)],
                                                         ktl[:, qsl], start=True, stop=True)
                                    nc.scalar.activation(ptile[rb][:, 1024 * h:1024 * (h + 1)], en[:],
                                                         AF.Exp, bias=e60[:])
                                    for _ in range(2 - h):
                                        if queue and queue[0][0] < GRB * g:
                                            emit_one(*queue.pop(0))
                                nc.vector.reduce_sum(rowsum[:, rb:rb + 1], ptile[rb][:],
                                                     axis=mybir.AxisListType.X)
                            gsl = slice(GRB * g, GRB * (g + 1))
                            nc.sync.dma_start(rs_in[g][:], rowsum[:, gsl])
                            if collective:
                                nc.gpsimd.collective_compute(
                                    "AllReduce", mybir.AluOpType.add,
                                    replica_groups=REPLICA_PAIRS,
                                    ins=[rs_in[g][:]], outs=[rs_out[g][:]],
                                )
                            else:
                                nc.sync.dma_start(rs_out[g][:], rs_in[g][:])
                            nc.sync.dma_start(rsf[:, gsl], rs_out[g][:])
                            nc.vector.reciprocal(recips[:, gsl], rsf[:, gsl])
                            for rb in range(GRB * g, GRB * (g + 1)):
                                nc.vector.tensor_scalar_mul(xvt[rb][:], xvt[rb][:], recips[:, rb:rb + 1])
                                for i in range(2):
                                    for hf in range(2):
                                        queue.append((rb, i, hf))
                        pEn_cm.__exit__(None, None, None)
                        for rb, i, hf in queue:          # drain leftovers
                            emit_one(rb, i, hf)
                        for i in range(2):
                            stg = stA.tile([128, 1024], F32, name="stg", tag="stg", bufs=2)
                            nc.scalar.copy(stg[:], nm0[i][:])
                            nc.sync.dma_start(nm_d[0][i][:], stg[:])

                # ---- tail: colsum(mg0) first, then numer(mg1)+colsum(mg1);
                #      finalize(mg0) chains hide under the mg1 matmuls ----
                with tc.tile_pool(name="s3", bufs=1) as s3:
                  with tc.tile_pool(name="pB2", bufs=1, space="PSUM") as pB2:
                    nm1 = [pB2.tile([128, 1024], F32, name=f"g1nm{i}", tag=f"g1nm{i}") for i in range(2)]
                    nmc1 = pB2.tile([1, 1024], F32, name="nmc1", tag="nmc1")
                    with tc.tile_pool(name="pB1", bufs=1, space="PSUM") as pB1:
                        nmc0 = pB1.tile([1, 1024], F32, name="nmc0", tag="nmc0")
                        for rb in range(RB):
                            st, sp = rb == 0, rb == RB - 1
                            for hf in range(2):
                                osl = slice(512 * hf, 512 * (hf + 1))
                                nc.tensor.matmul(nmc0[:, osl], xvt[rb][:, 256:257],
                                                 ptile[rb][:, osl], start=st, stop=sp)

                        def nm0_src(i, ssl):
                            m = s3.tile([128, 512], F32, name=f"nm0t{i}", tag=f"nm0t{i}", bufs=2)
                            nc.sync.dma_start(m[:], nm_d[0][i][:, ssl])
                            return m[:]

                        df0 = finalize_chains(nc, 0, nm0_src, lambda: nmc0[:], s3, x_loc)
                    with tc.tile_pool(name="pT", bufs=1, space="PSUM") as pT:
                        for rb in range(RB):
                            st, sp = rb == 0, rb == RB - 1
                            for hf in range(2):
                                osl = slice(512 * hf, 512 * (hf + 1))
                                psl = slice(1024 + 512 * hf, 1024 + 512 * (hf + 1))
                                nc.tensor.matmul(nmc1[:, osl], xvt[rb][:, 256:257], ptile[rb][:, psl], start=st, stop=sp)
                        # mg1 rc chain: nmc1 complete, runs under nm1 MMs
                        rcs1 = []
                        for sub in range(2):
                            ssl = slice(512 * sub, 512 * (sub + 1))
                            rc0 = s3.tile([1, 512], F32, name="rc1", tag="rc0", bufs=2)
                            nc.vector.tensor_scalar_add(rc0[:], nmc1[:, ssl], 1e-9)
                            nc.vector.reciprocal(rc0[:], rc0[:])
                            rcs = s3.tile([128, 512], F32, name="rcs1", tag="rcs", bufs=2)
                            nc.gpsimd.partition_broadcast(rcs[:], rc0[:])
                            rcs1.append(rcs)

                        def df1_chain(sub, bufs):
                            msl = slice(1024 + 512 * sub, 1024 + 512 * (sub + 1))
                            ssl = slice(512 * sub, 512 * (sub + 1))
                            out = {}
                            for i in range(2):
                                t = s3.tile([128, 512], F32, name=f"cxl{i}", tag=f"cxl{i}", bufs=2)
                                nc.sync.dma_start(t[:], x_loc[128 * i:128 * (i + 1), msl])
                                d = s3.tile([128, 512], F32R, name=f"df{i}", tag=f"df{i}", bufs=bufs)
                                nc.vector.tensor_mul(d[:], nm1[i][:, ssl], rcs1[sub][:])
                                nc.vector.tensor_sub(d[:], t[:], d[:])
                                out[(sub, i)] = d
                            return out

                        # column-half 0 of nm1 first: its diff chain then runs
                        # under the half-1 matmuls
                        for hf in range(2):
                            osl = slice(512 * hf, 512 * (hf + 1))
                            psl = slice(1024 + 512 * hf, 1024 + 512 * (hf + 1))
                            for rb in range(RB):
                                st, sp = rb == 0, rb == RB - 1
                                nc.tensor.matmul(nm1[0][:, osl], xvt[rb][:, 0:128], ptile[rb][:, psl], start=st, stop=sp)
                                nc.tensor.matmul(nm1[1][:, osl], xvt[rb][:, 128:256], ptile[rb][:, psl], start=st, stop=sp)
                            if hf == 0:
                                df1 = df1_chain(0, 3)
                        df1.update(df1_chain(1, 3))
                        finalize_tf(nc, 0, df0, pT, s3, x_loc, out_loc, w3t, bias3)
                        finalize_tf(nc, 1, df1, pT, s3, x_loc, out_loc, w3t, bias3)

    nc.compile()
    return nc


def make_in_maps(inputs):
    """Per-core input dicts from the full problem inputs."""
    g = {k: np.asarray(v, dtype=np.float32) for k, v in inputs.items()}
    x = g["x"]
    bn3_s, bn3_b = g["bn3_s"], g["bn3_b"]
    an_s, an_b = g["an_s"], g["an_b"]

    qkwT = np.ascontiguousarray(g["qk_w"].T)                     # [256, 32]
    w2 = bn3_s[:, None] * g["v_w"]                               # [256, 256]
    w2t = np.zeros((C, 258), np.float32)
    w2t[:, :256] = w2.T
    b2e = np.zeros((1, 258), np.float32)
    b2e[0, :256] = bn3_s * g["v_b"] + bn3_b
    b2e[0, 256] = 1.0
    w3t = np.ascontiguousarray((an_s[:, None] * g["trans_w"]).T)  # [256, 256]
    bias3 = np.ascontiguousarray((an_s * g["trans_b"] + an_b).reshape(2, 128).T)
    consts = dict(
        qkwT=qkwT, w2t=w2t, b2e=b2e, w3t=w3t, bias3=bias3,
        bn1s=g["bn1_s"].reshape(CR, 1), bn1b=g["bn1_b"].reshape(CR, 1),
        bn2s=g["bn2_s"].reshape(CR, 1), bn2b=g["bn2_b"].reshape(CR, 1),
        ones1=np.ones((1, 128), np.float32),
    )
    in_maps = []
    for core in range(8):
        b, h = core // 2, core % 2
        in_maps.append(dict(
            x_full=np.ascontiguousarray(x[b]),
            x_loc=np.ascontiguousarray(x[b][:, NH * h:NH * (h + 1)]),
            **consts,
        ))
    return in_maps


_NC_CACHE = []


def kernel(**inputs) -> np.ndarray:
    in_maps = make_in_maps(inputs)
    if not _NC_CACHE:
        _NC_CACHE.append(build_nc())
    nc = _NC_CACHE[0]
    res = run_bass_kernel_spmd(nc, in_maps, list(range(8)))
    out = np.empty((B, C, N), np.float32)
    for core in range(8):
        b, h = core // 2, core % 2
        out[b, :, NH * h:NH * (h + 1)] = res.results[core]["out_loc"]
    return out


if __name__ == "__main__":
    rng = np.random.default_rng(0)
    fake = dict(
        x=rng.standard_normal((B, C, N)).astype(np.float32),
        qk_w=rng.standard_normal((CR, C)).astype(np.float32) / 16,
        v_w=rng.standard_normal((C, C)).astype(np.float32) / 16,
        v_b=rng.standard_normal(C).astype(np.float32) * 0.01,
        trans_w=rng.standard_normal((C, C)).astype(np.float32) / 16,
        trans_b=rng.standard_normal(C).astype(np.float32) * 0.01,
        bn1_s=1 + 0.1 * rng.standard_normal(CR).astype(np.float32),
        bn1_b=0.01 * rng.standard_normal(CR).astype(np.float32),
        bn2_s=1 + 0.1 * rng.standard_normal(CR).astype(np.float32),
        bn2_b=0.01 * rng.standard_normal(CR).astype(np.float32),
        bn3_s=1 + 0.1 * rng.standard_normal(C).astype(np.float32),
        bn3_b=0.01 * rng.standard_normal(C).astype(np.float32),
        an_s=1 + 0.1 * rng.standard_normal(C).astype(np.float32),
        an_b=0.01 * rng.standard_normal(C).astype(np.float32),
    )
    out = kernel(**fake)
    print("kernel ran, out shape", out.shape, "finite:", np.isfinite(out).all())
